# revision 16
# baseline (speedup 1.0000x reference)
"""Trainium2 Bass kernel for nn_Binary_module_44263932953138 (UCell/AMS gene-set
scoring module).

Sharding: genes split across 8 cores (B row-shard + x_rank/x_log2 column-shard,
weight replicated). Each core computes gs_c = B_c @ W locally, partial
R/bg_num/raw_num over its gene shard, then one AllReduce of the partial-sum
buffer; batchnorm + final projection computed (redundantly) on every core.

v2: host-side preprocessing moves all data-marshalling off the device:
  * W binarized on host; gene axis compacted to the union of genes that appear
    in any set (~56% of genes) for the B columns, W rows and x_rank columns
    (exact - dropped columns contribute 0).
  * B / x_rank / x_log2 shards pre-transposed on host so the device needs no
    PE transposes; converted to bf16 (halves DMA at the same 1 cycle/row PE
    rate as fp32r).
  * x_rank is sent as xr' = maxrank - min(x_rank, maxrank) (93% exact zeros,
    small magnitudes -> tiny bf16 error); UCell affine constants absorb the
    sign flip: R_UCell = (S' + n(n+1)/2) * inv_nmr  (+ const killed by BN).
  * final projection folds out_w/batchnorm into a single per-row affine so the
    full [56, batch] normalize pass is skipped.

Self-contained: hardcodes shapes from the problem spec.
"""
import sys

for _p in ("/opt/trn_rl_repo", "/root/.axon_site/_ro/trn_rl_repo"):
    if _p not in sys.path:
        sys.path.insert(0, _p)

import numpy as np

import bass_rust
import concourse.bass as bass
import concourse.mybir as mybir
import concourse.tile as tile
from concourse.bass_utils import run_bass_kernel_spmd
from concourse.masks import make_identity

# ---------------------------------------------------------------------------
# Workaround for this container's walrus: every TPB instruction here accepts at
# most ONE sync-wait command, but Tile's sem assignment can attach several
# (e.g. the end-of-kernel drain, or a DMA waiting on multiple producers).
# Post-pass: hoist excess waits onto injected same-engine NoOps placed
# immediately before the instruction (the engine executes its stream in order,
# so wait-then-instruction semantics are preserved; for HWDGE DMAs this turns
# a queue-level wait into an issue-time wait, which is strictly stronger).
from concourse.tile import TileContext


def _split_multi_waits(nc, max_waits=1):
    for f in nc.m.functions:
        new_blocks = []
        for bb in f.blocks:
            rebuilt = []
            changed = False
            for ins in bb.instructions:
                si = ins.sync_info
                if si is not None and si.on_wait and len(si.on_wait) > max_waits:
                    waits = list(si.on_wait)
                    for w in waits[:-max_waits]:
                        nop = mybir.InstNoOp(
                            name=f"waitsplit-{nc.next_id()}", ins=[], outs=[]
                        )
                        nop.engine = ins.engine
                        nop.sync_info = bass_rust.SyncInfo(
                            on_wait=[w], on_update=[]
                        )
                        rebuilt.append(nop)
                    ins.sync_info = bass_rust.SyncInfo(
                        on_wait=waits[-max_waits:], on_update=list(si.on_update)
                    )
                    changed = True
                rebuilt.append(ins)
            if changed:
                nbb = bass_rust.BasicBlock(name=bb.name, instructions=rebuilt)
                nbb.IsExit = bb.IsExit
                nbb.IsLoopEntry = bb.IsLoopEntry
                nbb.IsPredicated = bb.IsPredicated
                new_blocks.append(nbb)
            else:
                new_blocks.append(bb)
        f.blocks = new_blocks
# ---------------------------------------------------------------------------

F32 = mybir.dt.float32
BF16 = mybir.dt.bfloat16
F8 = mybir.dt.float8e4
N_CORES = 8
P = 128

# Full problem config
G_REAL = 14271     # real genes
H = 24             # gene sets
BATCH = 4096
EPS = 1e-5
MAXRANK_PARAM = 1000.0
GSH = -(-G_REAL // (N_CORES * P)) * P   # per-core genes (1792), mult of 128
GT = GSH // P                           # gene tiles per core (14)

# partial-sum buffer layout (partition rows; 32-aligned engine bases):
#   rows 0:24   S'.T  (reversed-clamp rank sums)
#   rows 32:56  bg_num.T
#   rows 64:88  raw_num.T
PR_ROWS = 88


def build_nc(jut, kt, batch=BATCH, n_cores=N_CORES):
    """Build the SPMD Bass program (identical on all cores; per-core data
    differs via inputs).  jut = union-gene 128-tiles (contraction for B@W),
    kt = per-core compacted x_rank gene 128-tiles."""
    gsh = GSH
    assert batch % 1024 == 0
    NBC = batch // 1024                 # batch chunks of 1024
    CH1792 = [(0, 512), (512, 512), (1024, 512), (1536, 256)]
    CH1024 = [(0, 512), (512, 512)]

    nc = bass.Bass(num_devices=n_cores)
    bsh = nc.declare_dram_parameter("bsh", [jut * P, gsh], F8, isOutput=False)
    xlt = nc.declare_dram_parameter("xlt", [gsh, batch], BF16, isOutput=False)
    xrt = nc.declare_dram_parameter("xrt", [kt * P, batch], BF16, isOutput=False)
    wu = nc.declare_dram_parameter("wu", [jut * P, H], F8, isOutput=False)
    # wxr pre-scaled by inv_nmr on host; wsh by inv_n
    wxr = nc.declare_dram_parameter("wxr", [kt * P, H], BF16, isOutput=False)
    wsh = nc.declare_dram_parameter("wsh", [gsh, H], BF16, isOutput=False)
    # tcr: cols 0:24 tconst=(n+1)/(2*mr); cols 24:48 inv_gs (host-computed)
    tcr = nc.declare_dram_parameter("tcr", [1, 2 * H], F32, isOutput=False)
    ow = nc.declare_dram_parameter("ow", [1, 2 * H], F32, isOutput=False)
    ob = nc.declare_dram_parameter("ob", [1, 1], F32, isOutput=False)
    pred = nc.declare_dram_parameter("pred", [batch], F32, isOutput=True)

    PW = batch

    with TileContext(nc) as tc:
        with (
            tc.tile_pool(name="singles", bufs=1) as singles,
            tc.tile_pool(name="bpool", bufs=4) as bpool,
            tc.tile_pool(name="xpool", bufs=8) as xpool,
            tc.tile_pool(name="small", bufs=2) as small,
            tc.tile_pool(name="dram", bufs=1, space="DRAM") as dram,
        ):
            # ---------------- phase 0: prelude -----------------------------
            id128 = singles.tile([P, P], F32)
            make_identity(nc, id128)
            eps_sb = singles.tile([P, 1], F32)
            nc.vector.memset(eps_sb, EPS)

            # W (union genes, binary, fp8): [128, jut, 24]; per-core compact
            # W*inv_nmr for x_rank: [128, kt, 24]; per-core [gs | W_shard/n]:
            # [128, GT, 48]
            w_sb = singles.tile([P, jut, H], F8)
            nc.sync.dma_start(
                out=w_sb, in_=wu[:].rearrange("(t p) h -> p t h", p=P)
            )
            wxr_sb = singles.tile([P, kt, H], BF16)
            nc.sync.dma_start(
                out=wxr_sb, in_=wxr[:].rearrange("(t p) h -> p t h", p=P)
            )
            wg_sb = singles.tile([P, GT, 2 * H], BF16)
            nc.sync.dma_start(
                out=wg_sb[:, :, H:2 * H],
                in_=wsh[:].rearrange("(t p) h -> p t h", p=P),
            )

            # partial-sums buffer (transposed layout), all-reduced later
            part_sb = singles.tile([PR_ROWS, PW], F32)
            nc.vector.memset(part_sb[:], 0.0)

            # per-partition scalars, rows 0:24 (col 2 inv_gs, col 3 tconst)
            sc = singles.tile([56, 8], F32)
            nc.sync.dma_start(out=sc[0:H, 3:4], in_=tcr[0:1, 0:H])
            nc.sync.dma_start(out=sc[0:H, 2:3], in_=tcr[0:1, H:2 * H])

            # R_all.T: rows 0:24 = UCell, rows 32:56 = AMS.  Constructed
            # quarter-by-quarter as all-reduce results land.
            rall = singles.tile([56, batch], F32)
            nc.vector.memset(rall[:], 0.0)
            nsub = batch // 512
            stats = small.tile([56, nsub, 6], F32, tag="bnstats", name="stats")
            mv = small.tile([56, 2], F32, tag="bnaggr", name="mv")
            rstd = small.tile([56, 1], F32, tag="rstd", name="rstd")

            # ---------------- phase 1: gsT = (B_c @ W).T -------------------
            with (
                tc.tile_pool(name="psA", bufs=1, space="PSUM") as psA,
                tc.tile_pool(name="psB", bufs=2, space="PSUM") as psB,
            ):
                psum_gsT = psA.tile([H, gsh], F32)
                for j in range(jut):
                    bn = bpool.tile([P, gsh], F8, tag="bt", name="bn")
                    nc.sync.dma_start(out=bn, in_=bsh[j * P:(j + 1) * P, :])
                    for (c0, cw) in CH1792:
                        nc.tensor.matmul(
                            psum_gsT[:, c0:c0 + cw],
                            lhsT=w_sb[:, j, :], rhs=bn[:, c0:c0 + cw],
                            start=(j == 0), stop=(j == jut - 1),
                        )
                # epilogue: scale gsT by inv_gs (host-computed) during the
                # PSUM read, then transpose into WG[:, :, 0:24]; bg partials
                # then come out of the lg matmul pre-normalized.
                gsT_sb = small.tile([H, gsh], F32, tag="gsT_sb", name="gsT_sb")
                nc.vector.tensor_scalar(
                    out=gsT_sb, in0=psum_gsT, scalar1=sc[0:H, 2:3],
                    scalar2=None, op0=mybir.AluOpType.mult,
                )
                for t in range(GT):
                    ptile = psB.tile([P, H], F32, tag="tr", name="ptile")
                    nc.tensor.transpose(
                        ptile, gsT_sb[:, t * P:(t + 1) * P], id128[0:H, 0:H],
                    )
                    nc.any.tensor_copy(out=wg_sb[:, t, 0:H], in_=ptile)

            # collective buffers: all-reduce in 1024-wide quarters, first
            # three overlapped with phase 2
            sum_sb = singles.tile([PR_ROWS, PW], F32)
            cc_bufs = {}
            for i in range(NBC):
                lo, hi = i * 1024, (i + 1) * 1024
                cc_bufs[lo] = (
                    dram.tile([72, hi - lo], F32, name=f"cc_in{i}"),
                    dram.tile([72, hi - lo], F32, addr_space="Shared",
                              name=f"cc_out{i}"),
                )

            def emit_allreduce(lo, hi):
                # pack the three 24-row groups (drop alignment-gap rows)
                ci, co = cc_bufs[lo]
                nc.sync.dma_start(out=ci[0:24, :], in_=part_sb[0:24, lo:hi])
                nc.sync.dma_start(out=ci[24:48, :], in_=part_sb[32:56, lo:hi])
                nc.sync.dma_start(out=ci[48:72, :], in_=part_sb[64:88, lo:hi])
                nc.gpsimd.collective_compute(
                    "AllReduce", mybir.AluOpType.add,
                    replica_groups=[list(range(n_cores))],
                    ins=[ci[:]], outs=[co[:]],
                )
                # S' * inv_nmr is pre-folded; R_UCell = sum + tconst
                nc.sync.dma_start(out=sum_sb[0:24, lo:hi], in_=co[0:24, :])
                nc.vector.tensor_scalar(
                    out=rall[0:H, lo:hi], in0=sum_sb[0:H, lo:hi],
                    scalar1=sc[0:H, 3:4], scalar2=None,
                    op0=mybir.AluOpType.add,
                )
                # R_AMS = raw_scaled - bg_scaled (both pre-normalized)
                nc.sync.dma_start(out=rall[32:56, lo:hi], in_=co[48:72, :])
                nc.sync.dma_start(out=sum_sb[32:56, lo:hi], in_=co[24:48, :])
                nc.vector.tensor_tensor(
                    out=rall[32:56, lo:hi], in0=rall[32:56, lo:hi],
                    in1=sum_sb[32:56, lo:hi], op=mybir.AluOpType.subtract,
                )
                for s in range(lo // 512, hi // 512):
                    nc.vector.bn_stats(
                        out=stats[:, s, :], in_=rall[:, s * 512:(s + 1) * 512]
                    )

            # ---------------- phase 2: partial S' / bg / raw ---------------
            with tc.tile_pool(name="ps2", bufs=2, space="PSUM") as ps2:
                for bc in range(NBC):
                    b0 = bc * 1024
                    psum_lg = ps2.tile([2 * H, 1024], F32, tag="plg",
                                       name="psum_lg")
                    psum_r = ps2.tile([H, 1024], F32, tag="pr", name="psum_r")
                    for gt in range(GT):
                        xt = xpool.tile([P, 1024], BF16, tag="xt", name="xl_t")
                        nc.sync.dma_start(
                            out=xt, in_=xlt[gt * P:(gt + 1) * P, b0:b0 + 1024]
                        )
                        for (c0, cw) in CH1024:
                            nc.tensor.matmul(
                                psum_lg[:, c0:c0 + cw],
                                lhsT=wg_sb[:, gt, :], rhs=xt[:, c0:c0 + cw],
                                start=(gt == 0), stop=(gt == GT - 1),
                            )
                    for ktt in range(kt):
                        xt = xpool.tile([P, 1024], BF16, tag="xt", name="xr_t")
                        nc.sync.dma_start(
                            out=xt, in_=xrt[ktt * P:(ktt + 1) * P, b0:b0 + 1024]
                        )
                        for (c0, cw) in CH1024:
                            nc.tensor.matmul(
                                psum_r[:, c0:c0 + cw],
                                lhsT=wxr_sb[:, ktt, :], rhs=xt[:, c0:c0 + cw],
                                start=(ktt == 0), stop=(ktt == kt - 1),
                            )
                    nc.any.tensor_copy(
                        out=part_sb[0:H, b0:b0 + 1024], in_=psum_r
                    )
                    stage48 = small.tile([2 * H, 1024], F32, tag="stage48",
                                         name="stage48")
                    nc.any.tensor_copy(out=stage48, in_=psum_lg)
                    nc.sync.dma_start(
                        out=part_sb[32:56, b0:b0 + 1024], in_=stage48[0:H, :],
                    )
                    nc.sync.dma_start(
                        out=part_sb[64:88, b0:b0 + 1024], in_=stage48[H:2 * H, :],
                    )
                    if bc < NBC - 1:
                        emit_allreduce(b0, b0 + 1024)

            with tc.tile_pool(name="ps3", bufs=1, space="PSUM") as ps3:
                # ---------------- phase 3: all-reduce (last chunk) ---------
                emit_allreduce((NBC - 1) * 1024, PW)

                # ---------------- phase 4: final (redundant everywhere) ----
                # pred = sum_h a_h * (R_all[h] - mu_h) + (ob - 0) with
                # a = out_w * rstd; constant folded via c1 = sum_h a_h mu_h.
                nc.vector.bn_aggr(out=mv, in_=stats)
                nc.scalar.activation(
                    out=rstd, in_=mv[:, 1:2],
                    func=mybir.ActivationFunctionType.Sqrt,
                    bias=eps_sb[0:56], scale=1.0,
                )
                nc.vector.reciprocal(rstd, rstd)

                ow_sb = small.tile([1, 2 * H], F32, tag="ow", name="ow_sb")
                nc.sync.dma_start(out=ow_sb, in_=ow[:])
                owT = small.tile([56, 1], F32, tag="owT", name="owT")
                nc.vector.memset(owT, 0.0)
                nc.sync.dma_start(out=owT[0:H, 0:1], in_=ow_sb[0:1, 0:H])
                nc.sync.dma_start(out=owT[32:56, 0:1], in_=ow_sb[0:1, H:2 * H])
                ob_sb = small.tile([1, 1], F32, tag="ob", name="ob_sb")
                nc.sync.dma_start(out=ob_sb, in_=ob[:])

                av = small.tile([56, 1], F32, tag="av", name="av")
                nc.vector.tensor_tensor(
                    out=av, in0=owT, in1=rstd, op=mybir.AluOpType.mult
                )
                av_bf = small.tile([56, 1], BF16, tag="av_bf", name="av_bf")
                nc.vector.tensor_copy(out=av_bf, in_=av)
                # centered bf16 copy of R_all (one fused subtract+downcast)
                rb = small.tile([56, batch], BF16, tag="rb", name="rb")
                nc.vector.tensor_scalar(
                    out=rb, in0=rall[:], scalar1=mv[:, 0:1], scalar2=None,
                    op0=mybir.AluOpType.subtract,
                )
                for ch in range(batch // 512):
                    pp = ps3.tile([1, 512], F32, tag="pp", name="pp")
                    nc.tensor.matmul(
                        pp, lhsT=av_bf, rhs=rb[:, ch * 512:(ch + 1) * 512],
                        start=True, stop=True,
                    )
                    pout = small.tile([1, 512], F32, tag="pout", name="pout")
                    nc.vector.tensor_scalar(
                        out=pout, in0=pp, scalar1=ob_sb[0:1, 0:1],
                        scalar2=None, op0=mybir.AluOpType.add,
                    )
                    nc.sync.dma_start(
                        out=pred[ch * 512:(ch + 1) * 512], in_=pout[0:1, :]
                    )
    _split_multi_waits(nc)
    return nc


# ---------------------------------------------------------------------------
# host-side sharding + execution

_NC_CACHE = {}
_LAST_NC = None


def _get_nc(key):
    if key not in _NC_CACHE:
        _NC_CACHE[key] = build_nc(*key)
    return _NC_CACHE[key]


def shard_inputs(x_rank, x_log2, B, weight, maxrank_p, out_w, out_b):
    f32 = np.float32
    bf16 = mybir.dt.np(BF16)
    fp8 = mybir.dt.np(F8)
    gsh = GSH

    W = np.asarray(weight, f32) > 0                      # [G, H] bool
    Wf = W.astype(f32)
    n = Wf.sum(0, dtype=np.float64).astype(f32)          # exact small ints
    mrp0 = f32(np.asarray(maxrank_p, f32).reshape(-1)[0])
    maxrank = f32(f32(n.max() + f32(10.0)) + f32(max(mrp0, f32(0.0)) * f32(1000.0)))
    inv_nmr = (1.0 / (n.astype(np.float64) * np.float64(maxrank))).astype(f32)
    inv_n = (1.0 / n.astype(np.float64)).astype(f32)
    tconst = ((n.astype(np.float64) + 1.0) / (2.0 * np.float64(maxrank))).astype(f32)

    union = W.any(axis=1)
    uidx = np.flatnonzero(union)
    ju = len(uidx)
    jut = max(1, -(-ju // P))
    jup = jut * P

    core_idx = []
    for c in range(N_CORES):
        lo, hi = c * gsh, (c + 1) * gsh
        core_idx.append(uidx[(uidx >= lo) & (uidx < hi)])
    kmax = max(len(ic) for ic in core_idx)
    kt = max(1, -(-kmax // P))
    kp = kt * P

    # big conversions done once, then per-core strided transposes
    Bu = np.asarray(B, f32)[:, uidx].astype(fp8)         # [G, ju]
    wu_full = np.zeros((jup, H), fp8)
    wu_full[:ju] = Wf[uidx]
    xr2 = np.maximum(maxrank - np.asarray(x_rank, f32), 0.0)  # [B, G] f32
    xl = np.asarray(x_log2, f32)
    # gs column sums from the fp8-rounded B (matches device gs):
    # colsum(B@W)_h = colsum_B . W[:, h]
    csB = Bu.astype(f32).sum(axis=0, dtype=np.float64)   # [ju]
    gs_colsum = csB @ Wf[uidx].astype(np.float64)        # [H]
    inv_gs = (1.0 / gs_colsum).astype(f32)
    tcr_in = np.concatenate([tconst, inv_gs]).reshape(1, 2 * H)
    ow_in = np.asarray(out_w, f32).reshape(1, 2 * H)
    ob_in = np.asarray(out_b, f32).reshape(1, 1)

    in_maps = []
    for c in range(N_CORES):
        lo = c * gsh
        nre = max(0, min(lo + gsh, G_REAL) - lo)
        idx_c = core_idx[c]
        bshT = np.zeros((jup, gsh), fp8)
        bshT[:ju, :nre] = Bu[lo:lo + nre].T
        xltc = np.zeros((gsh, BATCH), bf16)
        xltc[:nre] = xl[:, lo:lo + nre].T
        xrtc = np.zeros((kp, BATCH), bf16)
        xrtc[:len(idx_c)] = xr2[:, idx_c].T
        wxrc = np.zeros((kp, H), bf16)
        wxrc[:len(idx_c)] = Wf[idx_c] * inv_nmr[None, :]
        wshc = np.zeros((gsh, H), bf16)
        wshc[:nre] = Wf[lo:lo + nre] * inv_n[None, :]
        in_maps.append({
            "bsh": bshT, "xlt": xltc, "xrt": xrtc,
            "wu": wu_full, "wxr": wxrc, "wsh": wshc,
            "tcr": tcr_in, "ow": ow_in, "ob": ob_in,
        })
    return in_maps, jut, kt


def kernel(x_rank, x_log2, B, weight, maxrank_p, out_w, out_b):
    global _LAST_NC
    in_maps, jut, kt = shard_inputs(
        x_rank, x_log2, B, weight, maxrank_p, out_w, out_b)
    nc = _get_nc((jut, kt))
    _LAST_NC = nc
    res = run_bass_kernel_spmd(nc, in_maps, core_ids=list(range(N_CORES)))
    return np.asarray(res.results[0]["pred"], np.float32).reshape(BATCH, 1)
